# revision 9
# baseline (speedup 1.0000x reference)
"""Trainium2 Bass kernel for nn_LogDetter: logdet(x.T @ x / n).

Strategy (per sharding hint): shard x row-wise across 8 NeuronCores.
Each core computes its local Gram matrix G_i = x_i.T @ x_i ([512, 512],
fp32 PSUM accumulation) on the TensorEngine; the host sums the per-core
Grams in float64 and takes the log-determinant.

Fast path ("fp8dr"): inputs are cast to fp8-e4m3 on the host and the
Gram runs as DoubleRow matmuls (2 fp8 weights per PE cell, 256-row
contraction per matmul) — ~1.5x the bf16/fp16 PE rate and half the DMA
bytes. Numerics that make this safe for a 2e-2 rel-err budget:
- only the block upper triangle is computed (the Gram is symmetric;
  the host mirrors it);
- the Gram DIAGONAL is recomputed exactly on the host in O(N*D)
  (sum x^2), erasing both the fp8 input-rounding error and the PE's
  DoubleRow truncation bias (measured -3e-5/row coherent on HW) on
  the diagonal;
- off-diagonal fp8 input-rounding error is zero-mean and contributes
  ~1e-3 absolute to the logdet (budget ~1.9e-2 absolute); the host
  feeds the corrected eigenvalues through the same fp32 log/sum
  pipeline the reference uses, which quantizes the result at ~5e-4
  relative — in practice reproducing the reference bit-exactly.
- accumulation is split into two PSUM groups (row halves) so the first
  half's PSUM drain + output DMA overlap the second half's matmuls.

Self-contained: hardcodes N=131072, D=512, 8 cores.
"""

import numpy as np

N_FULL = 131072
D = 512
N_CORES = 8
N_SHARD = N_FULL // N_CORES  # 16384
P = 128  # partition tile
COL_STARTS = [0, 128, 256, 384]  # per row-block m, first computed column
M_TILES = D // P  # 4

# MODE "fp8dr": fp8-e4m3 DoubleRow matmuls (256-row tiles), 2 PSUM groups
# MODE "fp16": single-group fp16 matmuls (128-row k-tiles) — the previous
#              champion (~90us); kept as fallback.
MODE = "fp8dr"
DEVICE_KW = dict(batch=2, bufs_x=12, warmup=5, out_dt="fp16")
DEVICE_KW_FP16 = dict(batch=2, bufs_x=12, bufs_c=1, input_dtype="fp16")

_cache = {}


def _build_nc_fp8dr(batch=1, bufs_x=16, warmup=8, out_dt="fp16"):
    """fp8-e4m3 DoubleRow Gram kernel: 64 tiles of 256 rows, two PSUM
    accumulation groups drained independently."""
    import concourse.bacc as bacc
    import concourse.mybir as mybir
    import concourse.tile as tile

    dt = mybir.dt
    odt = {"fp16": dt.float16, "fp32": dt.float32}[out_dt]
    nc = bacc.Bacc(
        "TRN2", target_bir_lowering=False, debug=False, num_devices=N_CORES
    )
    x = nc.dram_tensor("x", [N_SHARD, D], dt.float8e4, kind="ExternalInput").ap()
    g = nc.dram_tensor("gram", [2, D, D], odt, kind="ExternalOutput").ap()

    SUB = 2 * batch  # 128-row sub-tiles per DMA
    n_tiles = N_SHARD // 256  # 64 DoubleRow tiles
    n_dmas = n_tiles // batch
    # partition p holds SUB *consecutive* DRAM rows (SUB*p .. SUB*p+SUB-1):
    # each partition's DMA chunk is SUB*512 contiguous bytes (2KB at batch=2),
    # minimizing descriptor count. Any row->(p,slot) assignment is valid for
    # a Gram as long as the stationary and moving operands share it.
    x_t = x.rearrange("(j p s) d -> j p s d", p=P, s=SUB)
    t_group_end = (n_tiles // 2 - 1, n_tiles - 1)

    dr = mybir.MatmulPerfMode.DoubleRow

    with tile.TileContext(nc) as tc:
        with (
            tc.tile_pool(name="xin", bufs=bufs_x) as xin,
            tc.tile_pool(name="wsp", bufs=1) as wsp,
            tc.tile_pool(name="acc", bufs=1, space="PSUM") as accp,
            tc.tile_pool(name="gout", bufs=8) as gout,
        ):
            # 8 accumulators = 2 groups x 4 row blocks, one PSUM bank each
            accs = [
                [
                    accp.tile([P, D], dt.float32, name=f"acc{gi}_{m}", tag=f"acc{gi}{m}")
                    for m in range(M_TILES)
                ]
                for gi in range(2)
            ]
            if warmup:
                # dummy matmuls on zeroed SBUF: they depend on no DMA, so
                # they run right after the preamble and lift the PE HAM
                # clock-gate (1.2->2.4 GHz) before the first real matmul
                wsrc = wsp.tile([P, 2, D], dt.float8e4, name="wsrc", tag="wsrc")
                # DVE is idle in the preamble and its memset completes ~1.5us
                # earlier than GpSimd's would, so the warmup matmuls (and the
                # HAM un-throttle they trigger) start sooner
                nc.vector.memset(wsrc[:], 0.0)
                for w in range(warmup):
                    nc.tensor.matmul(
                        accs[0][0][:],
                        wsrc[:, :, :P],
                        wsrc[:],
                        start=True,
                        stop=True,
                        perf_mode=dr,
                        skip_group_check=True,
                    )
            for j in range(n_dmas):
                xt = xin.tile([P, SUB, D], dt.float8e4, name=f"x{j}", tag="x")
                if j == 0:
                    # split the first tile across sub-row halves and both DMA
                    # engines so the first data lands ~2x sooner
                    h = SUB // 2
                    nc.sync.dma_start(xt[:, :h], x_t[j][:, :h])
                    nc.scalar.dma_start(xt[:, h:], x_t[j][:, h:])
                else:
                    dma_eng = nc.sync if j % 2 == 0 else nc.scalar
                    dma_eng.dma_start(xt[:], x_t[j])
                for b in range(batch):
                    t = j * batch + b
                    gi = 0 if t <= t_group_end[0] else 1
                    first = t in (0, t_group_end[0] + 1)
                    last = t in t_group_end
                    # at the end of group 0, finish the small blocks first
                    # (their drain overlaps group 1's matmuls); at the end of
                    # group 1 finish the BIG block first so the final copy +
                    # DMA on the critical path is the smallest block
                    if last:
                        m_order = [3, 2, 1, 0] if gi == 0 else [0, 1, 2, 3]
                    else:
                        m_order = range(M_TILES)
                    for m in m_order:
                        cs = COL_STARTS[m]
                        nc.tensor.matmul(
                            accs[gi][m][:, : D - cs],
                            xt[:, 2 * b : 2 * b + 2, m * P : (m + 1) * P],
                            xt[:, 2 * b : 2 * b + 2, cs:D],
                            start=first,
                            stop=last,
                            perf_mode=dr,
                        )
                        if last:
                            ot = gout.tile(
                                [P, D - cs], odt, name=f"gsb{gi}_{m}", tag=f"g{gi}{m}"
                            )
                            # split drain work: copies alternate DVE/ACT;
                            # final-group DMA issues go on the two engines
                            # that are otherwise idle in the tail
                            if m % 2 == 0:
                                nc.vector.tensor_copy(ot[:], accs[gi][m][:, : D - cs])
                            else:
                                nc.scalar.copy(ot[:], accs[gi][m][:, : D - cs])
                            if gi == 0:
                                dma_out = nc.gpsimd
                            else:
                                dma_out = nc.gpsimd if m % 2 == 0 else nc.sync
                            dma_out.dma_start(g[gi, m * P : (m + 1) * P, cs:D], ot[:])
    nc.compile()
    return nc


def _build_nc_fp16(
    col_starts,
    bufs_x=16,
    bufs_c=12,
    cast_eng="dve",
    batch=1,
    dual_queue=False,
    input_dtype="fp16",
    warmup=0,
):
    """fp16 single-group Gram kernel (previous champion, fallback)."""
    import concourse.bacc as bacc
    import concourse.mybir as mybir
    import concourse.tile as tile

    dt = mybir.dt
    nc = bacc.Bacc(
        "TRN2", target_bir_lowering=False, debug=False, num_devices=N_CORES
    )
    in_dt = {"bf16": dt.bfloat16, "fp16": dt.float16, "fp32": dt.float32}[input_dtype]
    x = nc.dram_tensor("x", [N_SHARD, D], in_dt, kind="ExternalInput").ap()
    g = nc.dram_tensor("gram", [D, D], dt.float32, kind="ExternalOutput").ap()

    K_TILES = N_SHARD // P  # 128
    x_t = x.rearrange("(j two p) d -> j p two d", p=P, two=batch)
    n_batches = K_TILES // batch
    mm_dt = in_dt

    with tile.TileContext(nc) as tc:
        with (
            tc.tile_pool(name="xin", bufs=bufs_x) as xin,
            tc.tile_pool(name="acc", bufs=1, space="PSUM") as accp,
            tc.tile_pool(name="gout", bufs=2) as gout,
        ):
            accs = [
                accp.tile([P, D - col_starts[m]], dt.float32, name=f"acc{m}", tag=f"acc{m}")
                for m in range(M_TILES)
            ]
            for j in range(n_batches):
                xt = xin.tile([P, batch * D], in_dt, name=f"x{j}", tag="x")
                dma_eng = nc.sync if (not dual_queue or j % 2 == 0) else nc.scalar
                dma_eng.dma_start(
                    xt[:].rearrange("p (two d) -> p two d", d=D), x_t[j]
                )
                xmm = xt[:]
                for t in range(batch):
                    k = j * batch + t
                    first, last = k == 0, k == K_TILES - 1
                    base = t * D
                    m_order = range(M_TILES - 1, -1, -1) if last else range(M_TILES)
                    for m in m_order:
                        cs = col_starts[m]
                        nc.tensor.matmul(
                            accs[m][:],
                            xmm[:, base + m * P : base + (m + 1) * P],
                            xmm[:, base + cs : base + D],
                            start=first,
                            stop=last,
                        )
                        if last:
                            ot = gout.tile(
                                [P, D - cs], dt.float32, name=f"gsb{m}", tag=f"g{m}"
                            )
                            nc.vector.tensor_copy(ot[:], accs[m][:])
                            nc.sync.dma_start(g[m * P : (m + 1) * P, cs:D], ot[:])
    nc.compile()
    return nc


def _get_nc(mode=MODE, **kw):
    key = (mode, tuple(sorted(kw.items())))
    if key not in _cache:
        if mode == "fp8dr":
            _cache[key] = _build_nc_fp8dr(**kw)
        else:
            _cache[key] = _build_nc_fp16(COL_STARTS, **kw)
    return _cache[key]


def _run_device(x, mode=MODE, trace=False, **kw):
    """Run the 8-core Gram kernel. Returns (list of per-core gram arrays,
    BassKernelResults)."""
    import ml_dtypes
    from concourse.bass_utils import run_bass_kernel_spmd

    nc = _get_nc(mode, **kw)
    if mode == "fp8dr":
        x = x.astype(ml_dtypes.float8_e4m3)
    elif kw.get("input_dtype") == "bf16":
        x = x.astype(ml_dtypes.bfloat16)
    elif kw.get("input_dtype") == "fp16":
        x = x.astype(np.float16)
    shards = [
        np.ascontiguousarray(x[i * N_SHARD : (i + 1) * N_SHARD])
        for i in range(N_CORES)
    ]
    in_maps = [{"x": s} for s in shards]
    kwargs = {}
    if trace:
        kwargs = dict(trace=True, trace_cores=list(range(N_CORES)))
    res = run_bass_kernel_spmd(nc, in_maps, core_ids=list(range(N_CORES)), **kwargs)
    grams = [r["gram"] for r in res.results]
    return grams, res


def _logdet_from_grams(grams, x=None, mode=MODE):
    G = np.zeros((D, D), dtype=np.float64)
    for gm in grams:
        gm = np.asarray(gm, dtype=np.float64)
        if gm.ndim == 3:  # [2, D, D] group outputs
            gm = gm.sum(axis=0)
        G += gm
    # keep only the computed (block upper triangle) region, then mirror
    mask = np.zeros((D, D), dtype=bool)
    for m in range(M_TILES):
        mask[m * P : (m + 1) * P, COL_STARTS[m] :] = True
    G = np.where(mask, G, 0.0)
    U = np.triu(G)
    G = U + np.triu(G, 1).T
    if x is not None:
        # replace the diagonal with the exact sum(x^2): erases the fp8/fp16
        # input-rounding error and any device accumulation bias there
        x64 = x.astype(np.float64)
        G[np.arange(D), np.arange(D)] = np.einsum("nd,nd->d", x64, x64)
    # Mimic the reference's fp32 arithmetic exactly: it computes
    #   sum(2*log(svdvals_f32(x))) + d*(-log_f32(n))
    # in fp32, where both terms are ~6000 in magnitude — its own rounding
    # error is ~1e-3. Feeding our (more accurate) singular values through
    # the identical fp32 CPU-jax pipeline reproduces the reference's
    # quantization, typically bit-exactly.
    ev = np.linalg.eigvalsh(G)  # ascending; eig(x.T@x) = svdvals(x)**2
    s_f32 = np.sqrt(np.clip(ev[::-1], 1e-30, None)).astype(np.float32)
    try:
        import jax
        import jax.numpy as jnp

        with jax.default_device(jax.devices("cpu")[0]):
            val = jnp.sum(2.0 * jnp.log(jnp.asarray(s_f32))) + D * (
                -jnp.log(jnp.asarray(float(N_FULL), dtype=jnp.float32))
            )
            val = float(val)
        if not np.isfinite(val):
            raise FloatingPointError("mimic path produced non-finite value")
        return val
    except Exception:
        sign, logabsdet = np.linalg.slogdet(G / N_FULL)
        return float(logabsdet) if sign > 0 else float("nan")


def kernel(x):
    x = np.ascontiguousarray(np.asarray(x, dtype=np.float32))
    assert x.shape == (N_FULL, D), x.shape
    try:
        grams, _ = _run_device(x, **DEVICE_KW)
    except Exception:
        # one retry in case of a transient device/runtime hiccup
        grams, _ = _run_device(x, **DEVICE_KW)
    ld = _logdet_from_grams(grams, x=x)
    return np.asarray(ld, dtype=np.float32)


# revision 14
# speedup vs baseline: 1.0289x; 1.0289x over previous
"""Trainium2 Bass kernel for nn_LogDetter: logdet(x.T @ x / n).

Strategy (per sharding hint): shard x row-wise across 8 NeuronCores.
Each core computes its local Gram matrix G_i = x_i.T @ x_i ([512, 512],
fp32 PSUM accumulation) on the TensorEngine; the host sums the per-core
Grams in float64 and takes the log-determinant.

Fast path ("fp8dr"): inputs are cast to fp8-e4m3 on the host and the
Gram runs as DoubleRow matmuls (2 fp8 weights per PE cell, 256-row
contraction per matmul) — ~1.5x the bf16/fp16 PE rate and half the DMA
bytes. Numerics that make this safe for a 2e-2 rel-err budget:
- only the block upper triangle is computed (the Gram is symmetric;
  the host mirrors it);
- the Gram DIAGONAL is recomputed exactly on the host in O(N*D)
  (sum x^2), erasing both the fp8 input-rounding error and the PE's
  DoubleRow truncation bias (measured -3e-5/row coherent on HW) on
  the diagonal;
- off-diagonal fp8 input-rounding error is zero-mean and contributes
  ~1e-3 absolute to the logdet (budget ~1.9e-2 absolute); the host
  feeds the corrected eigenvalues through the same fp32 log/sum
  pipeline the reference uses, which quantizes the result at ~5e-4
  relative — in practice reproducing the reference bit-exactly.
- accumulation is split into two PSUM groups (row halves) so the first
  half's PSUM drain + output DMA overlap the second half's matmuls.

Self-contained: hardcodes N=131072, D=512, 8 cores.
"""

import numpy as np

N_FULL = 131072
D = 512
N_CORES = 8
N_SHARD = N_FULL // N_CORES  # 16384
P = 128  # partition tile
COL_STARTS = [0, 128, 256, 384]  # per row-block m, first computed column
M_TILES = D // P  # 4

# MODE "fp8dr": fp8-e4m3 DoubleRow matmuls (256-row tiles), 2 PSUM groups
# MODE "fp16": single-group fp16 matmuls (128-row k-tiles) — the previous
#              champion (~90us); kept as fallback.
MODE = "fp8dr"
DEVICE_KW = dict(batch=2, bufs_x=12, warmup=5, out_dt="fp16")
DEVICE_KW_FP16 = dict(batch=2, bufs_x=12, bufs_c=1, input_dtype="fp16")

_cache = {}


def _build_nc_fp8dr(batch=1, bufs_x=16, warmup=8, out_dt="fp16"):
    """fp8-e4m3 DoubleRow Gram kernel: 64 tiles of 256 rows, two PSUM
    accumulation groups drained independently."""
    import concourse.bacc as bacc
    import concourse.mybir as mybir
    import concourse.tile as tile

    dt = mybir.dt
    odt = {"fp16": dt.float16, "fp32": dt.float32}[out_dt]
    nc = bacc.Bacc(
        "TRN2", target_bir_lowering=False, debug=False, num_devices=N_CORES
    )
    x = nc.dram_tensor("x", [N_SHARD, D], dt.float8e4, kind="ExternalInput").ap()
    g = nc.dram_tensor("gram", [2, D, D], odt, kind="ExternalOutput").ap()

    SUB = 2 * batch  # 128-row sub-tiles per DMA
    n_tiles = N_SHARD // 256  # 64 DoubleRow tiles
    n_dmas = n_tiles // batch
    # partition p holds SUB *consecutive* DRAM rows (SUB*p .. SUB*p+SUB-1):
    # each partition's DMA chunk is SUB*512 contiguous bytes (2KB at batch=2),
    # minimizing descriptor count. Any row->(p,slot) assignment is valid for
    # a Gram as long as the stationary and moving operands share it.
    x_t = x.rearrange("(j p s) d -> j p s d", p=P, s=SUB)
    t_group_end = (n_tiles // 2 - 1, n_tiles - 1)

    dr = mybir.MatmulPerfMode.DoubleRow

    with tile.TileContext(nc) as tc:
        with (
            tc.tile_pool(name="xin", bufs=bufs_x) as xin,
            tc.tile_pool(name="wsp", bufs=1) as wsp,
            tc.tile_pool(name="acc", bufs=1, space="PSUM") as accp,
            tc.tile_pool(name="gout", bufs=8) as gout,
        ):
            # 8 accumulators = 2 groups x 4 row blocks, one PSUM bank each
            accs = [
                [
                    accp.tile([P, D], dt.float32, name=f"acc{gi}_{m}", tag=f"acc{gi}{m}")
                    for m in range(M_TILES)
                ]
                for gi in range(2)
            ]
            if warmup:
                # dummy matmuls on zeroed SBUF: they depend on no DMA, so
                # they run right after the preamble and lift the PE HAM
                # clock-gate (1.2->2.4 GHz) before the first real matmul
                wsrc = wsp.tile([P, 2, D], dt.float8e4, name="wsrc", tag="wsrc")
                # DVE is idle in the preamble and its memset completes ~1.5us
                # earlier than GpSimd's would, so the warmup matmuls (and the
                # HAM un-throttle they trigger) start sooner
                nc.vector.memset(wsrc[:], 0.0)
                for w in range(warmup):
                    nc.tensor.matmul(
                        accs[0][0][:],
                        wsrc[:, :, :P],
                        wsrc[:],
                        start=True,
                        stop=True,
                        perf_mode=dr,
                        skip_group_check=True,
                    )
            for j in range(n_dmas):
                xt = xin.tile([P, SUB, D], dt.float8e4, name=f"x{j}", tag="x")
                if j == 0 and SUB >= 3:
                    # first tile: 3-way split across all DMA-capable engines
                    # (parallel first issues + 3 queues) so it lands ~7.9us
                    # even under 8-core HBM startup contention — a late first
                    # tile idles the PE and postpones the HAM un-throttle,
                    # which costs ~2.5us on the affected core
                    nc.sync.dma_start(xt[:, :1], x_t[j][:, :1])
                    nc.scalar.dma_start(xt[:, 1:2], x_t[j][:, 1:2])
                    nc.gpsimd.dma_start(xt[:, 2:], x_t[j][:, 2:])
                elif j in (1, 2):
                    # next two tiles: 2-way splits, keeping arrival tight
                    h = SUB // 2
                    nc.sync.dma_start(xt[:, :h], x_t[j][:, :h])
                    nc.scalar.dma_start(xt[:, h:], x_t[j][:, h:])
                else:
                    dma_eng = nc.sync if j % 2 == 0 else nc.scalar
                    dma_eng.dma_start(xt[:], x_t[j])
                for b in range(batch):
                    t = j * batch + b
                    gi = 0 if t <= t_group_end[0] else 1
                    first = t in (0, t_group_end[0] + 1)
                    last = t in t_group_end
                    # at the end of group 0, finish the small blocks first
                    # (their drain overlaps group 1's matmuls); at the end of
                    # group 1 finish the BIG block first so the final copy +
                    # DMA on the critical path is the smallest block
                    if last:
                        m_order = [3, 2, 1, 0] if gi == 0 else [0, 1, 2, 3]
                    else:
                        m_order = range(M_TILES)
                    for m in m_order:
                        cs = COL_STARTS[m]
                        nc.tensor.matmul(
                            accs[gi][m][:, : D - cs],
                            xt[:, 2 * b : 2 * b + 2, m * P : (m + 1) * P],
                            xt[:, 2 * b : 2 * b + 2, cs:D],
                            start=first,
                            stop=last,
                            perf_mode=dr,
                        )
                        if last:
                            ot = gout.tile(
                                [P, D - cs], odt, name=f"gsb{gi}_{m}", tag=f"g{gi}{m}"
                            )
                            # split drain work: copies alternate DVE/ACT;
                            # final-group DMA issues go on the two engines
                            # that are otherwise idle in the tail
                            if m % 2 == 0:
                                nc.vector.tensor_copy(ot[:], accs[gi][m][:, : D - cs])
                            else:
                                nc.scalar.copy(ot[:], accs[gi][m][:, : D - cs])
                            if gi == 0:
                                nc.gpsimd.dma_start(
                                    g[gi, m * P : (m + 1) * P, cs:D], ot[:]
                                )
                            elif m == 0:
                                # final group's big block: halves on two queues
                                nc.gpsimd.dma_start(g[gi, 0:64, :], ot[0:64])
                                nc.gpsimd.dma_start(g[gi, 64:P, :], ot[64:P])
                            else:
                                # spread the remaining final-group issues over
                                # engines that are idle by then
                                dma_out = {1: nc.sync, 2: nc.scalar, 3: nc.sync}[m]
                                dma_out.dma_start(
                                    g[gi, m * P : (m + 1) * P, cs:D], ot[:]
                                )
    nc.compile()
    return nc


def _build_nc_fp16(
    col_starts,
    bufs_x=16,
    bufs_c=12,
    cast_eng="dve",
    batch=1,
    dual_queue=False,
    input_dtype="fp16",
    warmup=0,
):
    """fp16 single-group Gram kernel (previous champion, fallback)."""
    import concourse.bacc as bacc
    import concourse.mybir as mybir
    import concourse.tile as tile

    dt = mybir.dt
    nc = bacc.Bacc(
        "TRN2", target_bir_lowering=False, debug=False, num_devices=N_CORES
    )
    in_dt = {"bf16": dt.bfloat16, "fp16": dt.float16, "fp32": dt.float32}[input_dtype]
    x = nc.dram_tensor("x", [N_SHARD, D], in_dt, kind="ExternalInput").ap()
    g = nc.dram_tensor("gram", [D, D], dt.float32, kind="ExternalOutput").ap()

    K_TILES = N_SHARD // P  # 128
    x_t = x.rearrange("(j two p) d -> j p two d", p=P, two=batch)
    n_batches = K_TILES // batch
    mm_dt = in_dt

    with tile.TileContext(nc) as tc:
        with (
            tc.tile_pool(name="xin", bufs=bufs_x) as xin,
            tc.tile_pool(name="acc", bufs=1, space="PSUM") as accp,
            tc.tile_pool(name="gout", bufs=2) as gout,
        ):
            accs = [
                accp.tile([P, D - col_starts[m]], dt.float32, name=f"acc{m}", tag=f"acc{m}")
                for m in range(M_TILES)
            ]
            for j in range(n_batches):
                xt = xin.tile([P, batch * D], in_dt, name=f"x{j}", tag="x")
                dma_eng = nc.sync if (not dual_queue or j % 2 == 0) else nc.scalar
                dma_eng.dma_start(
                    xt[:].rearrange("p (two d) -> p two d", d=D), x_t[j]
                )
                xmm = xt[:]
                for t in range(batch):
                    k = j * batch + t
                    first, last = k == 0, k == K_TILES - 1
                    base = t * D
                    m_order = range(M_TILES - 1, -1, -1) if last else range(M_TILES)
                    for m in m_order:
                        cs = col_starts[m]
                        nc.tensor.matmul(
                            accs[m][:],
                            xmm[:, base + m * P : base + (m + 1) * P],
                            xmm[:, base + cs : base + D],
                            start=first,
                            stop=last,
                        )
                        if last:
                            ot = gout.tile(
                                [P, D - cs], dt.float32, name=f"gsb{m}", tag=f"g{m}"
                            )
                            nc.vector.tensor_copy(ot[:], accs[m][:])
                            nc.sync.dma_start(g[m * P : (m + 1) * P, cs:D], ot[:])
    nc.compile()
    return nc


def _get_nc(mode=MODE, **kw):
    key = (mode, tuple(sorted(kw.items())))
    if key not in _cache:
        if mode == "fp8dr":
            _cache[key] = _build_nc_fp8dr(**kw)
        else:
            _cache[key] = _build_nc_fp16(COL_STARTS, **kw)
    return _cache[key]


def _run_device(x, mode=MODE, trace=False, **kw):
    """Run the 8-core Gram kernel. Returns (list of per-core gram arrays,
    BassKernelResults)."""
    import ml_dtypes
    from concourse.bass_utils import run_bass_kernel_spmd

    nc = _get_nc(mode, **kw)
    if mode == "fp8dr":
        x = x.astype(ml_dtypes.float8_e4m3)
    elif kw.get("input_dtype") == "bf16":
        x = x.astype(ml_dtypes.bfloat16)
    elif kw.get("input_dtype") == "fp16":
        x = x.astype(np.float16)
    shards = [
        np.ascontiguousarray(x[i * N_SHARD : (i + 1) * N_SHARD])
        for i in range(N_CORES)
    ]
    in_maps = [{"x": s} for s in shards]
    kwargs = {}
    if trace:
        kwargs = dict(trace=True, trace_cores=list(range(N_CORES)))
    res = run_bass_kernel_spmd(nc, in_maps, core_ids=list(range(N_CORES)), **kwargs)
    grams = [r["gram"] for r in res.results]
    return grams, res


def _logdet_from_grams(grams, x=None, mode=MODE):
    G = np.zeros((D, D), dtype=np.float64)
    for gm in grams:
        gm = np.asarray(gm, dtype=np.float64)
        if gm.ndim == 3:  # [2, D, D] group outputs
            gm = gm.sum(axis=0)
        G += gm
    # keep only the computed (block upper triangle) region, then mirror
    mask = np.zeros((D, D), dtype=bool)
    for m in range(M_TILES):
        mask[m * P : (m + 1) * P, COL_STARTS[m] :] = True
    G = np.where(mask, G, 0.0)
    U = np.triu(G)
    G = U + np.triu(G, 1).T
    if x is not None:
        # replace the diagonal with the exact sum(x^2): erases the fp8/fp16
        # input-rounding error and any device accumulation bias there
        x64 = x.astype(np.float64)
        G[np.arange(D), np.arange(D)] = np.einsum("nd,nd->d", x64, x64)
    # Mimic the reference's fp32 arithmetic exactly: it computes
    #   sum(2*log(svdvals_f32(x))) + d*(-log_f32(n))
    # in fp32, where both terms are ~6000 in magnitude — its own rounding
    # error is ~1e-3. Feeding our (more accurate) singular values through
    # the identical fp32 CPU-jax pipeline reproduces the reference's
    # quantization, typically bit-exactly.
    ev = np.linalg.eigvalsh(G)  # ascending; eig(x.T@x) = svdvals(x)**2
    s_f32 = np.sqrt(np.clip(ev[::-1], 1e-30, None)).astype(np.float32)
    try:
        import jax
        import jax.numpy as jnp

        with jax.default_device(jax.devices("cpu")[0]):
            val = jnp.sum(2.0 * jnp.log(jnp.asarray(s_f32))) + D * (
                -jnp.log(jnp.asarray(float(N_FULL), dtype=jnp.float32))
            )
            val = float(val)
        if not np.isfinite(val):
            raise FloatingPointError("mimic path produced non-finite value")
        return val
    except Exception:
        sign, logabsdet = np.linalg.slogdet(G / N_FULL)
        return float(logabsdet) if sign > 0 else float("nan")


def kernel(x):
    x = np.ascontiguousarray(np.asarray(x, dtype=np.float32))
    assert x.shape == (N_FULL, D), x.shape
    try:
        grams, _ = _run_device(x, **DEVICE_KW)
    except Exception:
        # one retry in case of a transient device/runtime hiccup
        grams, _ = _run_device(x, **DEVICE_KW)
    ld = _logdet_from_grams(grams, x=x)
    return np.asarray(ld, dtype=np.float32)
